# revision 43
# baseline (speedup 1.0000x reference)
"""Trainium2 Bass kernel for a dense transformer block.

Reference math (B=32, S=577, D=768, H=12, DH=64, F=3072, fp32):
  h  = LN1(x);  q,k,v = per-head projections of h
  scores = q @ k^T / sqrt(DH)
  probs  = softmax(scores, axis=QUERY)       # quirk: softmax over the query axis
  attn   = probs @ v;  x2 = x + concat(attn) @ Wo + bo
  out    = x2 + (gelu(LN2(x2) @ W1 + b1) @ W2 + b2)

Strategy: pure data-parallel over batch, 4 batch items per core on 8 cores, no
collectives.  All on-chip activations live in a transposed layout [feature on
partitions, token on free dim], which makes every matmul contraction natural
and puts the quirky softmax on the free axis (row-wise) of the transposed
score matrices.  LN affine params are folded into the following weight matrix
on the host; biases are applied via per-partition scalars or rank-1 matmuls.

fp8 pipeline: all big GEMMs (QKV, Wo, FC1, FC2) run in fp8-e4m3 with DoubleRow
perf mode (2 fp8 MACs/PE-cell/cycle); probs@V runs plain fp8 (its 64-wide
stationary counts as column tiling, which is mutually exclusive with
DoubleRow).  The token axis is padded 577->640 so every matmul block is
512- or 128-wide (the 65-wide remainder would be LDWEIGHTS-bound).  Weights
are quantized host-side with a power-of-two scale folded into the post-matmul
dequant scalars; activations are written in fp8 directly by the DVE/ACT ops
that produce them.  LN stats run on early-emitted bf16 casts (DVE) and
GpSimd squares so the in-order PE queue never waits on them; the rstd rows
are partition-broadcast on GpSimd.  The head-pair loop is software-pipelined:
scores+exp of head-pair hp overlap probs@V of hp-1 plus one interleaved FC2
chunk of the previous batch item.  The residual stream stays fp32.
"""

import numpy as np
import ml_dtypes

B, S, D, H, DH, F = 32, 577, 768, 12, 64, 3072
NCORES = 8
BPC = B // NCORES          # batches per core
EPS = 1e-5
NCD = D // 128             # 6  d-chunks
NCF = F // 128             # 24 f-chunks
NHP = H // 2               # 6  head pairs
DSPL = [(0, 512), (512, D - 512)]              # free-dim splits of D
TCH = [(i * 128, min(128, S - i * 128)) for i in range((S + 127) // 128)]  # 5 t-chunks
SPAD = 640                 # padded token pitch: matmuls run 512+128-wide blocks
MSPL = [(0, 512), (512, 128)]                  # matmul splits over padded tokens

# fp8 quantization scales (powers of two; folded into dequant scalars)
SW = 64.0                  # all weight matrices
SZ = 8.0                   # z1/z2 (LN outputs)
SV = 8.0                   # v
SC = 32.0                  # attn concat

_NC_CACHE = {}


def _build_nc(gelu_kind: str = "gelu", bpc: int = BPC):
    from contextlib import ExitStack
    import concourse.bass as bass
    import concourse.tile as tile
    from concourse import bacc, mybir

    f32, bf16 = mybir.dt.float32, mybir.dt.bfloat16
    f8 = mybir.dt.float8e4
    AF = mybir.ActivationFunctionType
    ALU = mybir.AluOpType
    DR = mybir.MatmulPerfMode.DoubleRow
    GELU = {"gelu": AF.Gelu, "tanh": AF.Tanh}[gelu_kind]

    nc = bacc.Bacc("TRN2", target_bir_lowering=False, dynamic_dma_scratch_size=2048)
    xT_d = nc.declare_dram_parameter("xT", [bpc, D, S], f32, isOutput=False)
    wq_d = nc.declare_dram_parameter("wq", [D, D], f8, isOutput=False)
    wk_d = nc.declare_dram_parameter("wk", [D, D], f8, isOutput=False)
    wv_d = nc.declare_dram_parameter("wv", [D, D], f8, isOutput=False)
    wo_d = nc.declare_dram_parameter("wo", [D, D], f8, isOutput=False)
    w1_d = nc.declare_dram_parameter("w1", [D, F], f8, isOutput=False)
    w2_d = nc.declare_dram_parameter("w2", [F, D], f8, isOutput=False)
    bq_d = nc.declare_dram_parameter("bq", [NCD, 128], f32, isOutput=False)
    bk_d = nc.declare_dram_parameter("bk", [NCD, 128], f32, isOutput=False)
    bv_d = nc.declare_dram_parameter("bv", [1, D], bf16, isOutput=False)   # *SW*SZ
    bo_d = nc.declare_dram_parameter("bo", [1, D], bf16, isOutput=False)   # *SW*SC
    b1_d = nc.declare_dram_parameter("b1", [NCF, 128], f32, isOutput=False)
    b2_d = nc.declare_dram_parameter("b2", [1, D], bf16, isOutput=False)   # *SW
    outT_d = nc.declare_dram_parameter("outT", [bpc, D, S], f32, isOutput=True)

    with tile.TileContext(nc) as tc:
        with ExitStack() as ctx:
            wp = ctx.enter_context(tc.tile_pool(name="wp", bufs=1))
            rp = ctx.enter_context(tc.tile_pool(name="rp", bufs=2))      # residual f32
            zp = ctx.enter_context(tc.tile_pool(name="zp", bufs=1))      # normalized fp8
            qkp = ctx.enter_context(tc.tile_pool(name="qkp", bufs=1))    # qt/kt/v/concat
            ep = ctx.enter_context(tc.tile_pool(name="ep", bufs=4))      # exp tiles
            gp = ctx.enter_context(tc.tile_pool(name="gp", bufs=1))      # gelu acts
            sp_ = ctx.enter_context(tc.tile_pool(name="sp", bufs=1))     # small stat rows
            tp = ctx.enter_context(tc.tile_pool(name="tp", bufs=1))      # [128,*] temps
            bp = ctx.enter_context(tc.tile_pool(name="bp", bufs=2))      # bcast rows
            mmp = ctx.enter_context(tc.tile_pool(name="mmp", bufs=4, space="PSUM"))

            # ---- weights / constants (resident); DMAs deferred until after
            # the first x-shard load so compute starts immediately ----
            wq_s = wp.tile([128, NCD, D], f8, name="wq_s")
            wk_s = wp.tile([128, NCD, D], f8, name="wk_s")
            wv_s = wp.tile([128, NCD, D], f8, name="wv_s")
            wo_s = wp.tile([128, NCD, D], f8, name="wo_s")
            w1_s = wp.tile([128, NCD, F], f8, name="w1_s")
            w2_s = wp.tile([128, NCF, D], f8, name="w2_s")

            def emit_load_weights_qkv():
                nc.sync.dma_start(out=wq_s[:, :, :], in_=wq_d.ap().rearrange("(c p) n -> p c n", p=128))
                nc.sync.dma_start(out=wk_s[:, :, :], in_=wk_d.ap().rearrange("(c p) n -> p c n", p=128))
                nc.sync.dma_start(out=wv_s[:, :, :], in_=wv_d.ap().rearrange("(c p) n -> p c n", p=128))

            def emit_load_weights_rest():
                nc.sync.dma_start(out=wo_s[:, :, :], in_=wo_d.ap().rearrange("(c p) n -> p c n", p=128))
                nc.sync.dma_start(out=w1_s[:, :, :], in_=w1_d.ap().rearrange("(c p) n -> p c n", p=128))
                nc.sync.dma_start(out=w2_s[:, :, :], in_=w2_d.ap().rearrange("(c p) n -> p c n", p=128))
            bqs = wp.tile([128, NCD], f32, name="bqs")
            nc.sync.dma_start(out=bqs[:, :], in_=bq_d.ap().rearrange("c p -> p c"))
            bks = wp.tile([128, NCD], f32, name="bks")
            nc.sync.dma_start(out=bks[:, :], in_=bk_d.ap().rearrange("c p -> p c"))
            bvs = wp.tile([1, D], bf16, name="bvs")
            nc.sync.dma_start(out=bvs[:, :], in_=bv_d[:, :])
            bvs_bc = wp.tile([128, D], bf16, name="bvs_bc")
            nc.gpsimd.partition_broadcast(bvs_bc[:, :], bvs[0:1, :])
            bos = wp.tile([1, D], bf16, name="bos")
            nc.sync.dma_start(out=bos[:, :], in_=bo_d[:, :])
            b1s = wp.tile([128, NCF], f32, name="b1s")
            nc.sync.dma_start(out=b1s[:, :], in_=b1_d.ap().rearrange("c p -> p c"))
            b2s = wp.tile([1, D], bf16, name="b2s")
            nc.sync.dma_start(out=b2s[:, :], in_=b2_d[:, :])
            ones128 = wp.tile([128, 1], bf16, name="ones128")
            nc.vector.memset(ones128[:, :], 1.0)
            ones128f = wp.tile([128, 1], f32, name="ones128f")
            nc.vector.memset(ones128f[:, :], 1.0)
            ones1 = wp.tile([1, 640], bf16, name="ones1")
            nc.vector.memset(ones1[:, :], 1.0)
            eps_s = wp.tile([1, 1], f32, name="eps_s")
            nc.vector.memset(eps_s[:, :], EPS)
            eps_sz = wp.tile([1, 1], f32, name="eps_sz")
            nc.vector.memset(eps_sz[:, :], EPS / (SZ * SZ))
            kt0_64 = wp.tile([64, 64], f8, name="kt0_64")
            nc.vector.memset(kt0_64[:, :], 0.0)

            def emit_warm(n):
                """Write-only 512-wide matmuls on resident data: keep the PE
                array active through data-dependent stalls so the HAM clock
                gate never drops to K=4/8.  Results are never read."""
                warm = mmp.tile([128, SPAD], f32, name="warm", tag="mm")
                for _ in range(n):
                    nc.tensor.matmul(warm[0:64, 0:512], kt0_64[:, 0:64],
                                     wq_s[0:64, 0, 0:512], start=True, stop=True,
                                     skip_group_check=True)

            # ---------------- helpers ----------------
            def emit_xbsq(src, c, who):
                """bf16 cast (DVE, 2x SBUF mode) + GpSimd square of one chunk
                of fp32 src -> (xb, sq) bf16 tiles for the stats matmuls.
                Emitted EARLY (as soon as the chunk exists) so the in-order PE
                queue never stalls on them.  LN1 and LN2 use separate rings so
                their buffer reuse can never cross item boundaries."""
                xb = tp.tile([128, SPAD], bf16, name="xb" + who, tag="xb" + who, bufs=7)
                nc.vector.tensor_copy(xb[:, 0:S], src[:, c, 0:S])
                sq = tp.tile([128, SPAD], bf16, name="sq" + who, tag="sq" + who, bufs=7)
                nc.vector.tensor_mul(sq[:, 0:S], xb[:, 0:S], xb[:, 0:S])
                return xb, sq

            def emit_stats(pairs, warm=0):
                """Column sums & sums of squares over the partition (feature)
                axis -> psum rows [0]=sum, [32]=sumsq (bf16, 1 cyc/row).
                Token columns 577:640 are garbage and never read.  `warm`
                extra write-only matmuls into the unused row 64 keep the PE
                array active (HAM at K=8/8) through the LN-chain latency that
                gates the next consumer."""
                spt = mmp.tile([128, SPAD], f32, name="spt", tag="mm", padded_shape=[128, 1024])
                for c in range(NCD):
                    xb, sq = pairs[c]
                    for (s0, sn) in MSPL:
                        nc.tensor.matmul(spt[0:1, s0:s0 + sn], ones128[:, :],
                                         xb[:, s0:s0 + sn],
                                         start=(c == 0), stop=(c == NCD - 1))
                        nc.tensor.matmul(spt[32:33, s0:s0 + sn], ones128[:, :],
                                         sq[:, s0:s0 + sn],
                                         start=(c == 0), stop=(c == NCD - 1))
                for i in range(warm):
                    nc.tensor.matmul(spt[64:65, 0:512], ones128[:, :],
                                     pairs[i % NCD][0][:, 0:512],
                                     start=True, stop=True, skip_group_check=True)
                return spt

            def emit_chain(spt):
                """LN scalar chain on [1,S] rows (latency-optimized: DVE and
                ACT alternate; fast reciprocal via an SBUF bounce).  The fp8
                z-scale SZ is folded into rstd and -mu*rstd here."""
                mu_s = sp_.tile([1, S], f32, name="mu_s", tag="mu_s", bufs=2)
                nc.vector.tensor_scalar_mul(mu_s[:, :], spt[0:1, 0:S], -1.0 / D)
                msq = sp_.tile([1, S], f32, name="msq", tag="msq", bufs=2)
                nc.vector.tensor_mul(msq[:, :], mu_s[:, :], mu_s[:, :])
                v_s = sp_.tile([1, S], f32, name="v_s", tag="v_s", bufs=2)
                nc.vector.scalar_tensor_tensor(v_s[:, :], spt[32:33, 0:S], 1.0 / D,
                                               msq[:, :], op0=ALU.mult, op1=ALU.subtract)
                # sqrt((var + eps)/SZ^2): reciprocal then directly yields rstd*SZ
                w_s = sp_.tile([1, S], f32, name="w_s", tag="w_s", bufs=2)
                nc.scalar.activation(w_s[:, :], v_s[:, :], AF.Sqrt,
                                     scale=1.0 / (SZ * SZ), bias=eps_sz[0:1, 0:1])
                rc_s = sp_.tile([1, S], f32, name="rc_s", tag="rc_s", bufs=2)
                nc.vector.reciprocal_approx_fast(rc_s[:, :], w_s[:, :])
                rstd_bf = sp_.tile([1, S], bf16, name="rstd_bf", tag="rstdbf", bufs=2)
                nc.vector.tensor_scalar_mul(rstd_bf[:, :], rc_s[:, :], 1.0)
                nmr_bf = sp_.tile([1, S], bf16, name="nmr_bf", tag="nmrbf", bufs=2)
                nc.vector.tensor_mul(nmr_bf[:, :], mu_s[:, :], rc_s[:, :])
                return rstd_bf, nmr_bf

            def emit_bcast(row_bf, name):
                """Broadcast a [1,S] bf16 row across 128 partitions on GpSimd
                (keeps the PE free; lands in SBUF so znorm gets DVE 2x mode)."""
                bc = bp.tile([128, S], bf16, name=name, tag=name)
                nc.gpsimd.partition_broadcast(bc[:, :], row_bf[0:1, :])
                return bc

            def emit_znorm(src, rstd_bc, nmr_bc, z):
                """z[:,c,:] = (src[:,c,:] * rstd*SZ) + (-mu*rstd*SZ)  (fp8 out)."""
                for c in range(NCD):
                    zt = tp.tile([128, S], bf16, name="zt", tag="zt", bufs=2)
                    nc.vector.tensor_mul(zt[:, :], src[:, c, 0:S], rstd_bc[:, :])
                    nc.vector.tensor_add(z[:, c, 0:S], zt[:, :], nmr_bc[:, :])

            def emit_cmm(psum, w_t, x_t, wsl, s0, sn, first, last):
                """Contraction over NCD chunk pairs: DoubleRow fp8 on >=128-wide
                column blocks, plain fp8 (FWL) below that."""
                if sn >= 128:
                    for j in range(NCD // 2):
                        nc.tensor.matmul(psum[:, s0:s0 + sn],
                                         w_t[:, 2 * j:2 * j + 2, wsl],
                                         x_t[:, 2 * j:2 * j + 2, s0:s0 + sn],
                                         start=(first and j == 0),
                                         stop=(last and j == NCD // 2 - 1),
                                         perf_mode=DR)
                else:
                    for c in range(NCD):
                        nc.tensor.matmul(psum[:, s0:s0 + sn],
                                         w_t[:, c, wsl],
                                         x_t[:, c, s0:s0 + sn],
                                         start=(first and c == 0),
                                         stop=(last and c == NCD - 1))

            # ---------------- phase emitters ----------------
            xt_tiles = [None] * bpc
            z1_tiles = [None] * bpc
            z2_tiles = [None] * bpc
            x2_tiles = [None] * bpc
            st1 = [None] * bpc
            ch1 = [None] * bpc
            ch2 = [None] * bpc

            def emit_dma_x(b):
                xt = rp.tile([128, NCD, S], f32, name="xt", tag="res")
                for c in range(NCD):
                    nc.sync.dma_start(out=xt[:, c, :],
                                      in_=xT_d[b, c * 128:(c + 1) * 128, :])
                xt_tiles[b] = xt

            def emit_xprep(b):
                xt = xt_tiles[b]
                st1[b] = [emit_xbsq(xt, c, "1") for c in range(NCD)]

            def emit_stats1(b, warm=0):
                ch1[b] = emit_chain(emit_stats(st1[b], warm=warm))

            def emit_zfinish1(b):
                rstd_bf, nmr_bf = ch1[b]
                rbc = emit_bcast(rstd_bf, "rbc")
                nbc = emit_bcast(nmr_bf, "nbc")
                z1 = zp.tile([128, NCD, SPAD], f8, name="z1", tag="z")
                emit_znorm(xt_tiles[b], rbc, nbc, z1)
                z1_tiles[b] = z1

            qkv_tiles = [None] * bpc

            def emit_qkv(b):
                """QKV projections, emitted early so they fill the PE while
                the previous item's LN2 chain + znorm run on DVE/ACT/GpSimd."""
                z1 = z1_tiles[b]
                # --- QKV projections (fp8 DoubleRow; dequant+bias fused in
                # one DVE tensor_scalar) ---
                qt = qkp.tile([128, NHP, SPAD], bf16, name="qt", tag="qt")
                kt = qkp.tile([128, NHP, SPAD], bf16, name="kt", tag="kt")
                dqk = 1.0 / (SW * SZ)
                for hp in range(NHP):
                    hc = slice(hp * 128, (hp + 1) * 128)
                    qps = mmp.tile([128, SPAD], f32, name="qps", tag="mm")
                    for (s0, sn) in MSPL:
                        emit_cmm(qps, wq_s, z1, hc, s0, sn, True, True)
                    nc.vector.tensor_scalar(qt[:, hp, 0:S], qps[:, 0:S],
                                            dqk, bqs[:, hp:hp + 1],
                                            op0=ALU.mult, op1=ALU.add)
                    kps = mmp.tile([128, SPAD], f32, name="kps", tag="mm")
                    for (s0, sn) in MSPL:
                        emit_cmm(kps, wk_s, z1, hc, s0, sn, True, True)
                    nc.vector.tensor_scalar(kt[:, hp, 0:S], kps[:, 0:S],
                                            dqk, bks[:, hp:hp + 1],
                                            op0=ALU.mult, op1=ALU.add)
                # V in natural layout [t, v]; bias via rank-1 matmul (staged
                # *SW*SZ so it dequants together with the products)
                v = qkp.tile([128, len(TCH), D], f8, name="v", tag="v")
                for it, (t0, tw) in enumerate(TCH):
                    vps = mmp.tile([128, D], f32, name="vps", tag="mm")
                    for (s0, sn) in DSPL:
                        for j in range(NCD // 2):
                            nc.tensor.matmul(vps[0:tw, s0:s0 + sn],
                                             z1[:, 2 * j:2 * j + 2, t0:t0 + tw],
                                             wv_s[:, 2 * j:2 * j + 2, s0:s0 + sn],
                                             start=(j == 0), stop=(j == NCD // 2 - 1),
                                             perf_mode=DR)
                    nc.vector.scalar_tensor_tensor(v[0:tw, it, :], vps[0:tw, 0:D],
                                                   SV / (SW * SZ), bvs_bc[0:tw, 0:D],
                                                   op0=ALU.mult, op1=ALU.add)
                qkv_tiles[b] = (qt, kt, v)

            def emit_heads(b, interleave=()):
                """Software-pipelined head-pair loop: scores+exp(hp) overlap
                probs@V+concat(hp-1); `interleave` callables (FC2 chunks of
                the previous item) fill remaining PE gaps.  Ends with the
                output projection + LN2 stats prep."""
                interleave = list(interleave)
                qt, kt, v = qkv_tiles[b]
                concat = qkp.tile([128, NCD, SPAD], f8, name="concat", tag="concat")

                def emit_scores_exp(hp):
                    etiles = [None, None]
                    rstiles = [None, None]
                    for h2 in range(2):
                        hb = h2 * 64
                        e = ep.tile([128, len(TCH), SPAD], f8, name="e", tag="e")
                        rs = sp_.tile([128, len(TCH)], f32, name="rs", tag="rs", bufs=4)
                        for it, (t0, tw) in enumerate(TCH):
                            stps = mmp.tile([128, SPAD], f32, name="stps", tag="mm")
                            for (s0, sn) in MSPL:
                                nc.tensor.matmul(stps[0:tw, s0:s0 + sn],
                                                 kt[hb:hb + 64, hp, t0:t0 + tw],
                                                 qt[hb:hb + 64, hp, s0:s0 + sn],
                                                 start=True, stop=True)
                            nc.scalar.activation(e[0:tw, it, 0:S], stps[0:tw, 0:S], AF.Exp,
                                                 bias=0.0, scale=float(1.0 / np.sqrt(DH)))
                            # row sums on DVE: attention-phase DVE has slack
                            # while ACT is saturated; dropping accum_out kills
                            # the serial ACTIVATION_READ_ACCUMULATOR (~297ns/exp)
                            nc.vector.tensor_reduce(rs[0:tw, it:it + 1],
                                                    e[0:tw, it, 0:S],
                                                    mybir.AxisListType.X, ALU.add)
                        etiles[h2] = e
                        rstiles[h2] = rs
                    return etiles, rstiles

                def emit_attnv(hp, etiles, rstiles):
                    ap_ps = mmp.tile([128, SPAD], f32, name="ap_ps", tag="mm",
                                     padded_shape=[128, 1024])
                    for h2 in range(2):
                        hb = h2 * 64
                        e, rs = etiles[h2], rstiles[h2]
                        rec = sp_.tile([128, len(TCH)], f32, name="rec", tag="rec", bufs=2)
                        nfull = len(TCH) - 1
                        nc.vector.reciprocal(rec[:, 0:nfull], rs[:, 0:nfull])
                        lt0, ltw = TCH[-1]
                        nc.vector.reciprocal(rec[0:ltw, nfull:nfull + 1],
                                             rs[0:ltw, nfull:nfull + 1])
                        for it, (t0, tw) in enumerate(TCH):
                            nc.vector.tensor_scalar_mul(v[0:tw, it, hp * 128 + hb:hp * 128 + hb + 64],
                                                        v[0:tw, it, hp * 128 + hb:hp * 128 + hb + 64],
                                                        rec[0:tw, it:it + 1])
                        # probs @ v: plain fp8 (DoubleRow is illegal with the
                        # 64-wide stationary here — it counts as col tiling)
                        for (s0, sn) in MSPL:
                            for it, (t0, tw) in enumerate(TCH):
                                nc.tensor.matmul(ap_ps[hb:hb + 64, s0:s0 + sn],
                                                 v[0:tw, it, hp * 128 + hb:hp * 128 + hb + 64],
                                                 e[0:tw, it, s0:s0 + sn],
                                                 start=(it == 0), stop=(it == len(TCH) - 1))
                    nc.vector.tensor_scalar_mul(concat[:, hp, 0:S], ap_ps[:, 0:S], SC / SV)

                prev = None
                for hp in range(NHP):
                    cur = (hp, *emit_scores_exp(hp))
                    if prev is not None:
                        emit_attnv(*prev)
                        if interleave:
                            interleave.pop(0)()
                    prev = cur
                emit_attnv(*prev)
                if interleave:
                    interleave.pop(0)()

                # --- output projection + residual; LN2 stats prep (casts +
                # squares) runs per chunk, the stats matmuls afterwards so the
                # PE queue never waits on them (x re-read chunk-wise from DRAM
                # so the big x tile was released after znorm) ---
                x2 = rp.tile([128, NCD, SPAD], f32, name="x2", tag="res")
                pairs = []
                for ec in range(NCD):
                    xres = tp.tile([128, S], f32, name="xres", tag="xres", bufs=2)
                    nc.sync.dma_start(out=xres[:, :],
                                      in_=xT_d[b, ec * 128:(ec + 1) * 128, :])
                    wops = mmp.tile([128, SPAD], f32, name="wops", tag="mm")
                    ecs = slice(ec * 128, (ec + 1) * 128)
                    for (s0, sn) in MSPL:
                        nc.tensor.matmul(wops[:, s0:s0 + sn], bos[0:1, ecs],
                                         ones1[0:1, s0:s0 + sn], start=True, stop=False)
                        emit_cmm(wops, wo_s, concat, ecs, s0, sn, False, True)
                    nc.vector.scalar_tensor_tensor(x2[:, ec, 0:S], wops[:, 0:S],
                                                   1.0 / (SW * SC), xres[:, :],
                                                   op0=ALU.mult, op1=ALU.add)
                    pairs.append(emit_xbsq(x2, ec, "2"))
                x2_tiles[b] = x2
                return pairs

            def emit_stats2(b, pairs):
                ch2[b] = emit_chain(emit_stats(pairs))

            def emit_zfinish2(b):
                rstd_bf, nmr_bf = ch2[b]
                rbc = emit_bcast(rstd_bf, "rbc")
                nbc = emit_bcast(nmr_bf, "nbc")
                z2 = zp.tile([128, NCD, SPAD], f8, name="z2", tag="z")
                emit_znorm(x2_tiles[b], rbc, nbc, z2)
                z2_tiles[b] = z2

            def emit_fc1(b, g, lo, hi):
                z2 = z2_tiles[b]
                for fc in range(lo, hi):
                    fps = mmp.tile([128, SPAD], f32, name="fps", tag="mm")
                    fcs = slice(fc * 128, (fc + 1) * 128)
                    for (s0, sn) in MSPL:
                        emit_cmm(fps, w1_s, z2, fcs, s0, sn, True, True)
                    nc.scalar.activation(g[:, fc, 0:S], fps[:, 0:S], GELU,
                                         bias=b1s[:, fc:fc + 1], scale=1.0 / (SW * SZ))

            def emit_fc2_chunk(b, g, ec):
                x2 = x2_tiles[b]
                p2 = mmp.tile([128, SPAD], f32, name="p2", tag="mm")
                ecs = slice(ec * 128, (ec + 1) * 128)
                for (s0, sn) in MSPL:
                    nc.tensor.matmul(p2[:, s0:s0 + sn], b2s[0:1, ecs],
                                     ones1[0:1, s0:s0 + sn], start=True, stop=False)
                    for j in range(NCF // 2):
                        nc.tensor.matmul(p2[:, s0:s0 + sn],
                                         w2_s[:, 2 * j:2 * j + 2, ecs],
                                         g[:, 2 * j:2 * j + 2, s0:s0 + sn],
                                         start=False, stop=(j == NCF // 2 - 1),
                                         perf_mode=DR)
                nc.vector.scalar_tensor_tensor(x2[:, ec, 0:S], p2[:, 0:S],
                                               1.0 / SW, x2[:, ec, 0:S],
                                               op0=ALU.mult, op1=ALU.add)
                nc.sync.dma_start(out=outT_d[b, ec * 128:(ec + 1) * 128, :],
                                  in_=x2[:, ec, 0:S])

            # ---------------- emission schedule ----------------
            # Two-deep pipeline: FC2(b) chunks are interleaved into
            # attention(b+1)'s head-pair pipeline; chain1(b+1) hides under
            # FC1(b); LN2 stats prep is fused into the Wo chunk loop.
            # Rolling pipeline.  Per iteration (steady state):
            #   heads(b)   — attention + Wo; DVE slack absorbs xprep(b+1)
            #   stats2(b)  — chain2(b) on DVE/ACT rows
            #   stats1(b+1)— PE filler whose inputs are already ready, covers
            #                the chain2(b)+znorm2(b) latency window
            #   zfinish2(b), fc1(b)
            #   zfinish1(b+1), qkv(b+1) — PE keeps going after FC1
            emit_dma_x(0)
            emit_load_weights_qkv()
            emit_load_weights_rest()
            emit_xprep(0)
            emit_stats1(0)          # chain1(0) — exposed at startup only
            emit_zfinish1(0)
            emit_qkv(0)
            prev_fc2 = None
            for b in range(bpc):
                if b + 1 < bpc:
                    emit_dma_x(b + 1)
                    emit_xprep(b + 1)
                pairs2 = emit_heads(b, interleave=(prev_fc2 or []))
                emit_stats2(b, pairs2)
                emit_zfinish2(b)
                if b + 1 < bpc:
                    emit_stats1(b + 1, warm=52)
                g = gp.tile([128, NCF, SPAD], f8, name="g", tag="g")
                emit_fc1(b, g, 0, NCF)
                if b + 1 < bpc:
                    emit_zfinish1(b + 1)
                    emit_qkv(b + 1)
                prev_fc2 = [(lambda ec=ec, b=b, g=g: emit_fc2_chunk(b, g, ec))
                            for ec in range(NCD)]
            for ec in range(NCD):
                emit_fc2_chunk(bpc - 1, g, ec)
    nc.finalize()
    return nc


def _get_nc(gelu_kind: str = "gelu", bpc: int = BPC):
    key = (gelu_kind, bpc)
    if key not in _NC_CACHE:
        _NC_CACHE[key] = _build_nc(gelu_kind, bpc)
    return _NC_CACHE[key]


def _q8(a):
    return np.ascontiguousarray(
        np.clip(a * SW, -240.0, 240.0).astype(ml_dtypes.float8_e4m3))


def _prep_weights(inputs):
    bf16 = ml_dtypes.bfloat16
    f32 = np.float32
    Wq, Wk, Wv = inputs["Wq"], inputs["Wk"], inputs["Wv"]
    g1, b1_ln = np.asarray(inputs["ln1_g"], f32), np.asarray(inputs["ln1_b"], f32)
    g2, b2_ln = np.asarray(inputs["ln2_g"], f32), np.asarray(inputs["ln2_b"], f32)

    def flat(Wx):  # [H, D, DH] -> [D, H*DH]
        return np.ascontiguousarray(np.transpose(np.asarray(Wx, f32), (1, 0, 2)).reshape(D, D))

    wq_f, wk_f, wv_f = flat(Wq), flat(Wk), flat(Wv)
    W1 = np.asarray(inputs["W1"], f32)
    out = {
        "wq": _q8(g1[:, None] * wq_f),
        "wk": _q8(g1[:, None] * wk_f),
        "wv": _q8(g1[:, None] * wv_f),
        "wo": _q8(np.asarray(inputs["Wo"], f32)),
        "w1": _q8(g2[:, None] * W1),
        "w2": _q8(np.asarray(inputs["W2"], f32)),
        "bq": (b1_ln @ wq_f + np.asarray(inputs["bq"], f32).reshape(-1)).reshape(NCD, 128).astype(f32),
        "bk": (b1_ln @ wk_f + np.asarray(inputs["bk"], f32).reshape(-1)).reshape(NCD, 128).astype(f32),
        "bv": ((b1_ln @ wv_f + np.asarray(inputs["bv"], f32).reshape(-1))
               * SV).reshape(1, D).astype(bf16),
        "bo": (np.asarray(inputs["bo"], f32) * (SW * SC)).reshape(1, D).astype(bf16),
        "b1": (b2_ln @ W1 + np.asarray(inputs["b1"], f32)).reshape(NCF, 128).astype(f32),
        "b2": (np.asarray(inputs["b2"], f32) * SW).reshape(1, D).astype(bf16),
    }
    return out


def kernel(**inputs) -> np.ndarray:
    from concourse.bass_utils import run_bass_kernel_spmd

    nc = _get_nc("gelu", BPC)
    w = _prep_weights(inputs)
    x = np.asarray(inputs["x"], np.float32)
    # shard over batch, transpose to [b, D, S] per core
    xT = np.ascontiguousarray(
        x.reshape(NCORES, BPC, S, D).swapaxes(2, 3))  # [8, BPC, D, S]
    in_maps = [dict(w, xT=xT[i]) for i in range(NCORES)]
    res = run_bass_kernel_spmd(nc, in_maps, core_ids=list(range(NCORES)))
    outs = [res.results[i]["outT"] for i in range(NCORES)]   # each [BPC, D, S]
    out = np.stack(outs, 0).swapaxes(2, 3).reshape(B, S, D)
    return np.ascontiguousarray(out.astype(np.float32))


# revision 44
# speedup vs baseline: 1.0212x; 1.0212x over previous
"""Trainium2 Bass kernel for a dense transformer block.

Reference math (B=32, S=577, D=768, H=12, DH=64, F=3072, fp32):
  h  = LN1(x);  q,k,v = per-head projections of h
  scores = q @ k^T / sqrt(DH)
  probs  = softmax(scores, axis=QUERY)       # quirk: softmax over the query axis
  attn   = probs @ v;  x2 = x + concat(attn) @ Wo + bo
  out    = x2 + (gelu(LN2(x2) @ W1 + b1) @ W2 + b2)

Strategy: pure data-parallel over batch, 4 batch items per core on 8 cores, no
collectives.  All on-chip activations live in a transposed layout [feature on
partitions, token on free dim], which makes every matmul contraction natural
and puts the quirky softmax on the free axis (row-wise) of the transposed
score matrices.  LN affine params are folded into the following weight matrix
on the host; biases are applied via per-partition scalars or rank-1 matmuls.

fp8 pipeline: all big GEMMs (QKV, Wo, FC1, FC2) run in fp8-e4m3 with DoubleRow
perf mode (2 fp8 MACs/PE-cell/cycle); probs@V runs plain fp8 (its 64-wide
stationary counts as column tiling, which is mutually exclusive with
DoubleRow).  The token axis is padded 577->640 so every matmul block is
512- or 128-wide (the 65-wide remainder would be LDWEIGHTS-bound).  Weights
are quantized host-side with a power-of-two scale folded into the post-matmul
dequant scalars; activations are written in fp8 directly by the DVE/ACT ops
that produce them.  LN stats run on early-emitted bf16 casts (DVE) and
GpSimd squares so the in-order PE queue never waits on them; the rstd rows
are partition-broadcast on GpSimd.  The head-pair loop is software-pipelined:
scores+exp of head-pair hp overlap probs@V of hp-1 plus one interleaved FC2
chunk of the previous batch item.  The residual stream stays fp32.
"""

import numpy as np
import ml_dtypes

B, S, D, H, DH, F = 32, 577, 768, 12, 64, 3072
NCORES = 8
BPC = B // NCORES          # batches per core
EPS = 1e-5
NCD = D // 128             # 6  d-chunks
NCF = F // 128             # 24 f-chunks
NHP = H // 2               # 6  head pairs
DSPL = [(0, 512), (512, D - 512)]              # free-dim splits of D
TCH = [(i * 128, min(128, S - i * 128)) for i in range((S + 127) // 128)]  # 5 t-chunks
SPAD = 640                 # padded token pitch: matmuls run 512+128-wide blocks
MSPL = [(0, 512), (512, 128)]                  # matmul splits over padded tokens

# fp8 quantization scales (powers of two; folded into dequant scalars)
SW = 64.0                  # all weight matrices
SZ = 8.0                   # z1/z2 (LN outputs)
SV = 8.0                   # v
SC = 32.0                  # attn concat

_NC_CACHE = {}


def _build_nc(gelu_kind: str = "gelu", bpc: int = BPC):
    from contextlib import ExitStack
    import concourse.bass as bass
    import concourse.tile as tile
    from concourse import bacc, mybir

    f32, bf16 = mybir.dt.float32, mybir.dt.bfloat16
    f8 = mybir.dt.float8e4
    AF = mybir.ActivationFunctionType
    ALU = mybir.AluOpType
    DR = mybir.MatmulPerfMode.DoubleRow
    GELU = {"gelu": AF.Gelu, "tanh": AF.Tanh}[gelu_kind]

    nc = bacc.Bacc("TRN2", target_bir_lowering=False, dynamic_dma_scratch_size=2048)
    xT_d = nc.declare_dram_parameter("xT", [bpc, D, S], f32, isOutput=False)
    wq_d = nc.declare_dram_parameter("wq", [D, D], f8, isOutput=False)
    wk_d = nc.declare_dram_parameter("wk", [D, D], f8, isOutput=False)
    wv_d = nc.declare_dram_parameter("wv", [D, D], f8, isOutput=False)
    wo_d = nc.declare_dram_parameter("wo", [D, D], f8, isOutput=False)
    w1_d = nc.declare_dram_parameter("w1", [D, F], f8, isOutput=False)
    w2_d = nc.declare_dram_parameter("w2", [F, D], f8, isOutput=False)
    bq_d = nc.declare_dram_parameter("bq", [NCD, 128], f32, isOutput=False)
    bk_d = nc.declare_dram_parameter("bk", [NCD, 128], f32, isOutput=False)
    bv_d = nc.declare_dram_parameter("bv", [1, D], bf16, isOutput=False)   # *SW*SZ
    bo_d = nc.declare_dram_parameter("bo", [1, D], bf16, isOutput=False)   # *SW*SC
    b1_d = nc.declare_dram_parameter("b1", [NCF, 128], f32, isOutput=False)
    b2_d = nc.declare_dram_parameter("b2", [1, D], bf16, isOutput=False)   # *SW
    outT_d = nc.declare_dram_parameter("outT", [bpc, D, S], f32, isOutput=True)

    with tile.TileContext(nc) as tc:
        with ExitStack() as ctx:
            wp = ctx.enter_context(tc.tile_pool(name="wp", bufs=1))
            rp = ctx.enter_context(tc.tile_pool(name="rp", bufs=2))      # residual f32
            zp = ctx.enter_context(tc.tile_pool(name="zp", bufs=1))      # normalized fp8
            qkp = ctx.enter_context(tc.tile_pool(name="qkp", bufs=1))    # qt/kt/v/concat
            ep = ctx.enter_context(tc.tile_pool(name="ep", bufs=4))      # exp tiles
            gp = ctx.enter_context(tc.tile_pool(name="gp", bufs=1))      # gelu acts
            sp_ = ctx.enter_context(tc.tile_pool(name="sp", bufs=1))     # small stat rows
            tp = ctx.enter_context(tc.tile_pool(name="tp", bufs=1))      # [128,*] temps
            bp = ctx.enter_context(tc.tile_pool(name="bp", bufs=2))      # bcast rows
            mmp = ctx.enter_context(tc.tile_pool(name="mmp", bufs=4, space="PSUM"))

            # ---- weights / constants (resident); DMAs deferred until after
            # the first x-shard load so compute starts immediately ----
            wq_s = wp.tile([128, NCD, D], f8, name="wq_s")
            wk_s = wp.tile([128, NCD, D], f8, name="wk_s")
            wv_s = wp.tile([128, NCD, D], f8, name="wv_s")
            wo_s = wp.tile([128, NCD, D], f8, name="wo_s")
            w1_s = wp.tile([128, NCD, F], f8, name="w1_s")
            w2_s = wp.tile([128, NCF, D], f8, name="w2_s")

            def emit_load_weights_qkv():
                nc.sync.dma_start(out=wq_s[:, :, :], in_=wq_d.ap().rearrange("(c p) n -> p c n", p=128))
                nc.sync.dma_start(out=wk_s[:, :, :], in_=wk_d.ap().rearrange("(c p) n -> p c n", p=128))
                nc.sync.dma_start(out=wv_s[:, :, :], in_=wv_d.ap().rearrange("(c p) n -> p c n", p=128))

            def emit_load_weights_rest():
                nc.sync.dma_start(out=wo_s[:, :, :], in_=wo_d.ap().rearrange("(c p) n -> p c n", p=128))
                nc.sync.dma_start(out=w1_s[:, :, :], in_=w1_d.ap().rearrange("(c p) n -> p c n", p=128))
                nc.sync.dma_start(out=w2_s[:, :, :], in_=w2_d.ap().rearrange("(c p) n -> p c n", p=128))
            bqs = wp.tile([128, NCD], f32, name="bqs")
            nc.sync.dma_start(out=bqs[:, :], in_=bq_d.ap().rearrange("c p -> p c"))
            bks = wp.tile([128, NCD], f32, name="bks")
            nc.sync.dma_start(out=bks[:, :], in_=bk_d.ap().rearrange("c p -> p c"))
            bvs = wp.tile([1, D], bf16, name="bvs")
            nc.sync.dma_start(out=bvs[:, :], in_=bv_d[:, :])
            bvs_bc = wp.tile([128, D], bf16, name="bvs_bc")
            nc.gpsimd.partition_broadcast(bvs_bc[:, :], bvs[0:1, :])
            bos = wp.tile([1, D], bf16, name="bos")
            nc.sync.dma_start(out=bos[:, :], in_=bo_d[:, :])
            b1s = wp.tile([128, NCF], f32, name="b1s")
            nc.sync.dma_start(out=b1s[:, :], in_=b1_d.ap().rearrange("c p -> p c"))
            b2s = wp.tile([1, D], bf16, name="b2s")
            nc.sync.dma_start(out=b2s[:, :], in_=b2_d[:, :])
            ones128 = wp.tile([128, 1], bf16, name="ones128")
            nc.vector.memset(ones128[:, :], 1.0)
            ones128f = wp.tile([128, 1], f32, name="ones128f")
            nc.vector.memset(ones128f[:, :], 1.0)
            ones1 = wp.tile([1, 640], bf16, name="ones1")
            nc.vector.memset(ones1[:, :], 1.0)
            eps_s = wp.tile([1, 1], f32, name="eps_s")
            nc.vector.memset(eps_s[:, :], EPS)
            eps_sz = wp.tile([1, 1], f32, name="eps_sz")
            nc.vector.memset(eps_sz[:, :], EPS / (SZ * SZ))
            kt0_64 = wp.tile([64, 64], f8, name="kt0_64")
            nc.vector.memset(kt0_64[:, :], 0.0)

            def emit_warm(n):
                """Write-only 512-wide matmuls on resident data: keep the PE
                array active through data-dependent stalls so the HAM clock
                gate never drops to K=4/8.  Results are never read."""
                warm = mmp.tile([128, SPAD], f32, name="warm", tag="mm")
                for _ in range(n):
                    nc.tensor.matmul(warm[0:64, 0:512], kt0_64[:, 0:64],
                                     wq_s[0:64, 0, 0:512], start=True, stop=True,
                                     skip_group_check=True)

            # ---------------- helpers ----------------
            def emit_xbsq(src, c, who):
                """bf16 cast (DVE, 2x SBUF mode) + GpSimd square of one chunk
                of fp32 src -> (xb, sq) bf16 tiles for the stats matmuls.
                Emitted EARLY (as soon as the chunk exists) so the in-order PE
                queue never stalls on them.  LN1 and LN2 use separate rings so
                their buffer reuse can never cross item boundaries."""
                xb = tp.tile([128, SPAD], bf16, name="xb" + who, tag="xb" + who, bufs=7)
                nc.vector.tensor_copy(xb[:, 0:S], src[:, c, 0:S])
                sq = tp.tile([128, SPAD], bf16, name="sq" + who, tag="sq" + who, bufs=7)
                nc.vector.tensor_mul(sq[:, 0:S], xb[:, 0:S], xb[:, 0:S])
                return xb, sq

            def emit_stats(pairs, warm=0):
                """Column sums & sums of squares over the partition (feature)
                axis -> psum rows [0]=sum, [32]=sumsq (bf16, 1 cyc/row).
                Token columns 577:640 are garbage and never read.  `warm`
                extra write-only matmuls into the unused row 64 keep the PE
                array active (HAM at K=8/8) through the LN-chain latency that
                gates the next consumer."""
                spt = mmp.tile([128, SPAD], f32, name="spt", tag="mm", padded_shape=[128, 1024])
                for c in range(NCD):
                    xb, sq = pairs[c]
                    for (s0, sn) in MSPL:
                        nc.tensor.matmul(spt[0:1, s0:s0 + sn], ones128[:, :],
                                         xb[:, s0:s0 + sn],
                                         start=(c == 0), stop=(c == NCD - 1))
                        nc.tensor.matmul(spt[32:33, s0:s0 + sn], ones128[:, :],
                                         sq[:, s0:s0 + sn],
                                         start=(c == 0), stop=(c == NCD - 1))
                for i in range(warm):
                    nc.tensor.matmul(spt[64:65, 0:512], ones128[:, :],
                                     pairs[i % NCD][0][:, 0:512],
                                     start=True, stop=True, skip_group_check=True)
                return spt

            def emit_chain(spt):
                """LN scalar chain on [1,S] rows (latency-optimized: DVE and
                ACT alternate; fast reciprocal via an SBUF bounce).  The fp8
                z-scale SZ is folded into rstd and -mu*rstd here."""
                mu_s = sp_.tile([1, S], f32, name="mu_s", tag="mu_s", bufs=2)
                nc.vector.tensor_scalar_mul(mu_s[:, :], spt[0:1, 0:S], -1.0 / D)
                msq = sp_.tile([1, S], f32, name="msq", tag="msq", bufs=2)
                nc.vector.tensor_mul(msq[:, :], mu_s[:, :], mu_s[:, :])
                v_s = sp_.tile([1, S], f32, name="v_s", tag="v_s", bufs=2)
                nc.vector.scalar_tensor_tensor(v_s[:, :], spt[32:33, 0:S], 1.0 / D,
                                               msq[:, :], op0=ALU.mult, op1=ALU.subtract)
                # sqrt((var + eps)/SZ^2): reciprocal then directly yields rstd*SZ
                w_s = sp_.tile([1, S], f32, name="w_s", tag="w_s", bufs=2)
                nc.scalar.activation(w_s[:, :], v_s[:, :], AF.Sqrt,
                                     scale=1.0 / (SZ * SZ), bias=eps_sz[0:1, 0:1])
                rc_s = sp_.tile([1, S], f32, name="rc_s", tag="rc_s", bufs=2)
                nc.vector.reciprocal_approx_fast(rc_s[:, :], w_s[:, :])
                rstd_bf = sp_.tile([1, S], bf16, name="rstd_bf", tag="rstdbf", bufs=2)
                nc.vector.tensor_scalar_mul(rstd_bf[:, :], rc_s[:, :], 1.0)
                nmr_bf = sp_.tile([1, S], bf16, name="nmr_bf", tag="nmrbf", bufs=2)
                nc.vector.tensor_mul(nmr_bf[:, :], mu_s[:, :], rc_s[:, :])
                return rstd_bf, nmr_bf

            def emit_bcast(row_bf, name):
                """Broadcast a [1,S] bf16 row across 128 partitions on GpSimd
                (keeps the PE free; lands in SBUF so znorm gets DVE 2x mode)."""
                bc = bp.tile([128, S], bf16, name=name, tag=name)
                nc.gpsimd.partition_broadcast(bc[:, :], row_bf[0:1, :])
                return bc

            def emit_znorm(src, rstd_bc, nmr_bc, z):
                """z[:,c,:] = (src[:,c,:] * rstd*SZ) + (-mu*rstd*SZ)  (fp8 out)."""
                for c in range(NCD):
                    zt = tp.tile([128, S], bf16, name="zt", tag="zt", bufs=2)
                    nc.vector.tensor_mul(zt[:, :], src[:, c, 0:S], rstd_bc[:, :])
                    nc.vector.tensor_add(z[:, c, 0:S], zt[:, :], nmr_bc[:, :])

            def emit_cmm(psum, w_t, x_t, wsl, s0, sn, first, last):
                """Contraction over NCD chunk pairs: DoubleRow fp8 on >=128-wide
                column blocks, plain fp8 (FWL) below that."""
                if sn >= 128:
                    for j in range(NCD // 2):
                        nc.tensor.matmul(psum[:, s0:s0 + sn],
                                         w_t[:, 2 * j:2 * j + 2, wsl],
                                         x_t[:, 2 * j:2 * j + 2, s0:s0 + sn],
                                         start=(first and j == 0),
                                         stop=(last and j == NCD // 2 - 1),
                                         perf_mode=DR)
                else:
                    for c in range(NCD):
                        nc.tensor.matmul(psum[:, s0:s0 + sn],
                                         w_t[:, c, wsl],
                                         x_t[:, c, s0:s0 + sn],
                                         start=(first and c == 0),
                                         stop=(last and c == NCD - 1))

            # ---------------- phase emitters ----------------
            xt_tiles = [None] * bpc
            z1_tiles = [None] * bpc
            z2_tiles = [None] * bpc
            x2_tiles = [None] * bpc
            st1 = [None] * bpc
            ch1 = [None] * bpc
            ch2 = [None] * bpc

            def emit_dma_x(b):
                xt = rp.tile([128, NCD, S], f32, name="xt", tag="res")
                for c in range(NCD):
                    nc.sync.dma_start(out=xt[:, c, :],
                                      in_=xT_d[b, c * 128:(c + 1) * 128, :])
                xt_tiles[b] = xt

            def emit_xprep(b):
                xt = xt_tiles[b]
                st1[b] = [emit_xbsq(xt, c, "1") for c in range(NCD)]

            def emit_stats1(b, warm=0):
                ch1[b] = emit_chain(emit_stats(st1[b], warm=warm))

            def emit_zfinish1(b):
                rstd_bf, nmr_bf = ch1[b]
                rbc = emit_bcast(rstd_bf, "rbc")
                nbc = emit_bcast(nmr_bf, "nbc")
                z1 = zp.tile([128, NCD, SPAD], f8, name="z1", tag="z")
                emit_znorm(xt_tiles[b], rbc, nbc, z1)
                z1_tiles[b] = z1

            qkv_tiles = [None] * bpc

            def emit_qkv(b):
                """QKV projections, emitted early so they fill the PE while
                the previous item's LN2 chain + znorm run on DVE/ACT/GpSimd."""
                z1 = z1_tiles[b]
                # --- QKV projections (fp8 DoubleRow; dequant+bias fused in
                # one DVE tensor_scalar) ---
                qt = qkp.tile([128, NHP, SPAD], bf16, name="qt", tag="qt")
                kt = qkp.tile([128, NHP, SPAD], bf16, name="kt", tag="kt")
                dqk = 1.0 / (SW * SZ)
                for hp in range(NHP):
                    hc = slice(hp * 128, (hp + 1) * 128)
                    qps = mmp.tile([128, SPAD], f32, name="qps", tag="mm")
                    for (s0, sn) in MSPL:
                        emit_cmm(qps, wq_s, z1, hc, s0, sn, True, True)
                    nc.vector.tensor_scalar(qt[:, hp, 0:S], qps[:, 0:S],
                                            dqk, bqs[:, hp:hp + 1],
                                            op0=ALU.mult, op1=ALU.add)
                    kps = mmp.tile([128, SPAD], f32, name="kps", tag="mm")
                    for (s0, sn) in MSPL:
                        emit_cmm(kps, wk_s, z1, hc, s0, sn, True, True)
                    nc.vector.tensor_scalar(kt[:, hp, 0:S], kps[:, 0:S],
                                            dqk, bks[:, hp:hp + 1],
                                            op0=ALU.mult, op1=ALU.add)
                # V in natural layout [t, v]; bias via rank-1 matmul (staged
                # *SW*SZ so it dequants together with the products)
                v = qkp.tile([128, len(TCH), D], f8, name="v", tag="v")
                for it, (t0, tw) in enumerate(TCH):
                    vps = mmp.tile([128, D], f32, name="vps", tag="mm")
                    for (s0, sn) in DSPL:
                        for j in range(NCD // 2):
                            nc.tensor.matmul(vps[0:tw, s0:s0 + sn],
                                             z1[:, 2 * j:2 * j + 2, t0:t0 + tw],
                                             wv_s[:, 2 * j:2 * j + 2, s0:s0 + sn],
                                             start=(j == 0), stop=(j == NCD // 2 - 1),
                                             perf_mode=DR)
                    nc.vector.scalar_tensor_tensor(v[0:tw, it, :], vps[0:tw, 0:D],
                                                   SV / (SW * SZ), bvs_bc[0:tw, 0:D],
                                                   op0=ALU.mult, op1=ALU.add)
                qkv_tiles[b] = (qt, kt, v)

            def emit_heads(b, interleave=()):
                """Software-pipelined head-pair loop: scores+exp(hp) overlap
                probs@V+concat(hp-1); `interleave` callables (FC2 chunks of
                the previous item) fill remaining PE gaps.  Ends with the
                output projection + LN2 stats prep."""
                interleave = list(interleave)
                qt, kt, v = qkv_tiles[b]
                concat = qkp.tile([128, NCD, SPAD], f8, name="concat", tag="concat")

                def emit_scores_exp(hp):
                    etiles = [None, None]
                    rstiles = [None, None]
                    for h2 in range(2):
                        hb = h2 * 64
                        e = ep.tile([128, len(TCH), SPAD], f8, name="e", tag="e")
                        rs = sp_.tile([128, len(TCH)], f32, name="rs", tag="rs", bufs=4)
                        for it, (t0, tw) in enumerate(TCH):
                            stps = mmp.tile([128, SPAD], f32, name="stps", tag="mm")
                            for (s0, sn) in MSPL:
                                nc.tensor.matmul(stps[0:tw, s0:s0 + sn],
                                                 kt[hb:hb + 64, hp, t0:t0 + tw],
                                                 qt[hb:hb + 64, hp, s0:s0 + sn],
                                                 start=True, stop=True)
                            nc.scalar.activation(e[0:tw, it, 0:S], stps[0:tw, 0:S], AF.Exp,
                                                 bias=0.0, scale=float(1.0 / np.sqrt(DH)))
                            # row sums on DVE: attention-phase DVE has slack
                            # while ACT is saturated; dropping accum_out kills
                            # the serial ACTIVATION_READ_ACCUMULATOR (~297ns/exp)
                            nc.vector.tensor_reduce(rs[0:tw, it:it + 1],
                                                    e[0:tw, it, 0:S],
                                                    mybir.AxisListType.X, ALU.add)
                        etiles[h2] = e
                        rstiles[h2] = rs
                    return etiles, rstiles

                def emit_attnv(hp, etiles, rstiles):
                    ap_ps = mmp.tile([128, SPAD], f32, name="ap_ps", tag="mm",
                                     padded_shape=[128, 1024])
                    for h2 in range(2):
                        hb = h2 * 64
                        e, rs = etiles[h2], rstiles[h2]
                        rec = sp_.tile([128, len(TCH)], f32, name="rec", tag="rec", bufs=2)
                        nfull = len(TCH) - 1
                        nc.vector.reciprocal(rec[:, 0:nfull], rs[:, 0:nfull])
                        lt0, ltw = TCH[-1]
                        nc.vector.reciprocal(rec[0:ltw, nfull:nfull + 1],
                                             rs[0:ltw, nfull:nfull + 1])
                        for it, (t0, tw) in enumerate(TCH):
                            nc.vector.tensor_scalar_mul(v[0:tw, it, hp * 128 + hb:hp * 128 + hb + 64],
                                                        v[0:tw, it, hp * 128 + hb:hp * 128 + hb + 64],
                                                        rec[0:tw, it:it + 1])
                        # probs @ v: plain fp8 (DoubleRow is illegal with the
                        # 64-wide stationary here — it counts as col tiling)
                        for (s0, sn) in MSPL:
                            for it, (t0, tw) in enumerate(TCH):
                                nc.tensor.matmul(ap_ps[hb:hb + 64, s0:s0 + sn],
                                                 v[0:tw, it, hp * 128 + hb:hp * 128 + hb + 64],
                                                 e[0:tw, it, s0:s0 + sn],
                                                 start=(it == 0), stop=(it == len(TCH) - 1))
                    nc.vector.tensor_scalar_mul(concat[:, hp, 0:S], ap_ps[:, 0:S], SC / SV)

                prev = None
                for hp in range(NHP):
                    cur = (hp, *emit_scores_exp(hp))
                    if prev is not None:
                        if hp == 1 and interleave:
                            # fill the pipeline-fill bubble: attnV(0) waits
                            # ~7us for the first exps; this chunk is ready now
                            interleave.pop(0)()
                        emit_attnv(*prev)
                        if interleave:
                            interleave.pop(0)()
                    prev = cur
                emit_attnv(*prev)
                if interleave:
                    interleave.pop(0)()

                # --- output projection + residual; LN2 stats prep (casts +
                # squares) runs per chunk, the stats matmuls afterwards so the
                # PE queue never waits on them (x re-read chunk-wise from DRAM
                # so the big x tile was released after znorm) ---
                x2 = rp.tile([128, NCD, SPAD], f32, name="x2", tag="res")
                pairs = []
                for ec in range(NCD):
                    xres = tp.tile([128, S], f32, name="xres", tag="xres", bufs=2)
                    nc.sync.dma_start(out=xres[:, :],
                                      in_=xT_d[b, ec * 128:(ec + 1) * 128, :])
                    wops = mmp.tile([128, SPAD], f32, name="wops", tag="mm")
                    ecs = slice(ec * 128, (ec + 1) * 128)
                    for (s0, sn) in MSPL:
                        nc.tensor.matmul(wops[:, s0:s0 + sn], bos[0:1, ecs],
                                         ones1[0:1, s0:s0 + sn], start=True, stop=False)
                        emit_cmm(wops, wo_s, concat, ecs, s0, sn, False, True)
                    nc.vector.scalar_tensor_tensor(x2[:, ec, 0:S], wops[:, 0:S],
                                                   1.0 / (SW * SC), xres[:, :],
                                                   op0=ALU.mult, op1=ALU.add)
                    pairs.append(emit_xbsq(x2, ec, "2"))
                x2_tiles[b] = x2
                return pairs

            def emit_stats2(b, pairs):
                ch2[b] = emit_chain(emit_stats(pairs))

            def emit_zfinish2(b):
                rstd_bf, nmr_bf = ch2[b]
                rbc = emit_bcast(rstd_bf, "rbc")
                nbc = emit_bcast(nmr_bf, "nbc")
                z2 = zp.tile([128, NCD, SPAD], f8, name="z2", tag="z")
                emit_znorm(x2_tiles[b], rbc, nbc, z2)
                z2_tiles[b] = z2

            def emit_fc1(b, g, lo, hi):
                z2 = z2_tiles[b]
                for fc in range(lo, hi):
                    fps = mmp.tile([128, SPAD], f32, name="fps", tag="mm")
                    fcs = slice(fc * 128, (fc + 1) * 128)
                    for (s0, sn) in MSPL:
                        emit_cmm(fps, w1_s, z2, fcs, s0, sn, True, True)
                    nc.scalar.activation(g[:, fc, 0:S], fps[:, 0:S], GELU,
                                         bias=b1s[:, fc:fc + 1], scale=1.0 / (SW * SZ))

            def emit_fc2_chunk(b, g, ec):
                x2 = x2_tiles[b]
                p2 = mmp.tile([128, SPAD], f32, name="p2", tag="mm")
                ecs = slice(ec * 128, (ec + 1) * 128)
                for (s0, sn) in MSPL:
                    nc.tensor.matmul(p2[:, s0:s0 + sn], b2s[0:1, ecs],
                                     ones1[0:1, s0:s0 + sn], start=True, stop=False)
                    for j in range(NCF // 2):
                        nc.tensor.matmul(p2[:, s0:s0 + sn],
                                         w2_s[:, 2 * j:2 * j + 2, ecs],
                                         g[:, 2 * j:2 * j + 2, s0:s0 + sn],
                                         start=False, stop=(j == NCF // 2 - 1),
                                         perf_mode=DR)
                nc.vector.scalar_tensor_tensor(x2[:, ec, 0:S], p2[:, 0:S],
                                               1.0 / SW, x2[:, ec, 0:S],
                                               op0=ALU.mult, op1=ALU.add)
                nc.sync.dma_start(out=outT_d[b, ec * 128:(ec + 1) * 128, :],
                                  in_=x2[:, ec, 0:S])

            # ---------------- emission schedule ----------------
            # Two-deep pipeline: FC2(b) chunks are interleaved into
            # attention(b+1)'s head-pair pipeline; chain1(b+1) hides under
            # FC1(b); LN2 stats prep is fused into the Wo chunk loop.
            # Rolling pipeline.  Per iteration (steady state):
            #   heads(b)   — attention + Wo; DVE slack absorbs xprep(b+1)
            #   stats2(b)  — chain2(b) on DVE/ACT rows
            #   stats1(b+1)— PE filler whose inputs are already ready, covers
            #                the chain2(b)+znorm2(b) latency window
            #   zfinish2(b), fc1(b)
            #   zfinish1(b+1), qkv(b+1) — PE keeps going after FC1
            emit_dma_x(0)
            emit_load_weights_qkv()
            emit_load_weights_rest()
            emit_xprep(0)
            emit_stats1(0)          # chain1(0) — exposed at startup only
            emit_zfinish1(0)
            emit_qkv(0)
            prev_fc2 = None
            for b in range(bpc):
                if b + 1 < bpc:
                    emit_dma_x(b + 1)
                    emit_xprep(b + 1)
                pairs2 = emit_heads(b, interleave=(prev_fc2 or []))
                emit_stats2(b, pairs2)
                emit_zfinish2(b)
                if b + 1 < bpc:
                    emit_stats1(b + 1, warm=52)
                g = gp.tile([128, NCF, SPAD], f8, name="g", tag="g")
                emit_fc1(b, g, 0, NCF)
                if b + 1 < bpc:
                    emit_zfinish1(b + 1)
                    emit_qkv(b + 1)
                prev_fc2 = [(lambda ec=ec, b=b, g=g: emit_fc2_chunk(b, g, ec))
                            for ec in range(NCD)]
            for ec in range(NCD):
                emit_fc2_chunk(bpc - 1, g, ec)
    nc.finalize()
    return nc


def _get_nc(gelu_kind: str = "gelu", bpc: int = BPC):
    key = (gelu_kind, bpc)
    if key not in _NC_CACHE:
        _NC_CACHE[key] = _build_nc(gelu_kind, bpc)
    return _NC_CACHE[key]


def _q8(a):
    return np.ascontiguousarray(
        np.clip(a * SW, -240.0, 240.0).astype(ml_dtypes.float8_e4m3))


def _prep_weights(inputs):
    bf16 = ml_dtypes.bfloat16
    f32 = np.float32
    Wq, Wk, Wv = inputs["Wq"], inputs["Wk"], inputs["Wv"]
    g1, b1_ln = np.asarray(inputs["ln1_g"], f32), np.asarray(inputs["ln1_b"], f32)
    g2, b2_ln = np.asarray(inputs["ln2_g"], f32), np.asarray(inputs["ln2_b"], f32)

    def flat(Wx):  # [H, D, DH] -> [D, H*DH]
        return np.ascontiguousarray(np.transpose(np.asarray(Wx, f32), (1, 0, 2)).reshape(D, D))

    wq_f, wk_f, wv_f = flat(Wq), flat(Wk), flat(Wv)
    W1 = np.asarray(inputs["W1"], f32)
    out = {
        "wq": _q8(g1[:, None] * wq_f),
        "wk": _q8(g1[:, None] * wk_f),
        "wv": _q8(g1[:, None] * wv_f),
        "wo": _q8(np.asarray(inputs["Wo"], f32)),
        "w1": _q8(g2[:, None] * W1),
        "w2": _q8(np.asarray(inputs["W2"], f32)),
        "bq": (b1_ln @ wq_f + np.asarray(inputs["bq"], f32).reshape(-1)).reshape(NCD, 128).astype(f32),
        "bk": (b1_ln @ wk_f + np.asarray(inputs["bk"], f32).reshape(-1)).reshape(NCD, 128).astype(f32),
        "bv": ((b1_ln @ wv_f + np.asarray(inputs["bv"], f32).reshape(-1))
               * SV).reshape(1, D).astype(bf16),
        "bo": (np.asarray(inputs["bo"], f32) * (SW * SC)).reshape(1, D).astype(bf16),
        "b1": (b2_ln @ W1 + np.asarray(inputs["b1"], f32)).reshape(NCF, 128).astype(f32),
        "b2": (np.asarray(inputs["b2"], f32) * SW).reshape(1, D).astype(bf16),
    }
    return out


def kernel(**inputs) -> np.ndarray:
    from concourse.bass_utils import run_bass_kernel_spmd

    nc = _get_nc("gelu", BPC)
    w = _prep_weights(inputs)
    x = np.asarray(inputs["x"], np.float32)
    # shard over batch, transpose to [b, D, S] per core
    xT = np.ascontiguousarray(
        x.reshape(NCORES, BPC, S, D).swapaxes(2, 3))  # [8, BPC, D, S]
    in_maps = [dict(w, xT=xT[i]) for i in range(NCORES)]
    res = run_bass_kernel_spmd(nc, in_maps, core_ids=list(range(NCORES)))
    outs = [res.results[i]["outT"] for i in range(NCORES)]   # each [BPC, D, S]
    out = np.stack(outs, 0).swapaxes(2, 3).reshape(B, S, D)
    return np.ascontiguousarray(out.astype(np.float32))
